# revision 1
# baseline (speedup 1.0000x reference)
"""DNeRF distortion MoE-routing kernel for 8 Trainium2 NeuronCores.

Strategy
--------
`times` partitions the N=131072 points into 8 classes; the reference runs all
8 per-class MLPs densely on every point and selects.  Here we route on the
host instead: stable-sort points by class, give class c to NeuronCore c
(counts are ~16384 each), and each core runs the 4-layer MLP
(3->256->256->256->3, tanh everywhere) exactly once per point.  That is 8x
less compute and needs no cross-device communication.  The host scatters the
per-core results back to the original point order.

Device kernel (identical SPMD program on all 8 cores)
-----------------------------------------------------
The per-core workload is tanh-bound: 771 tanh/point must go through the
scalar (ACT) engine at 1 elem/lane/cycle @1.2GHz, while the matmuls (fp16,
1 cycle/row) leave the PE engine ~20% slack.  The kernel is therefore
organized to keep ACT 100% busy with large activation instructions:

- Points are processed in waves of 2048 (4 chunks of 512).  Layer ell,
  M-half m of a wave fills one 4-bank PSUM set [128, 4, 512]; a single
  ACT instruction applies bias+tanh over the whole set into SBUF.
  (Per-(layer,m) phases keep the bias a per-partition [128,1] AP.)
- Two PSUM sets ping-pong; waves are software-pipelined in pairs so the
  PE always fills one set while ACT drains the other and ACT never waits
  on same-wave dependencies.
- Layer 1 (K=3): the 4 chunks of a wave run concurrently in the four
  32-row groups of the PE array (weights replicated at partition offsets
  0/32/64/96, tile_position=(32j,0)).
- Layer 4 (M=3): the 4 chunks pack into the four 32-column groups of one
  PSUM bank (tile_position=(0,32j)), so the final tanh is one [128,512]
  ACT instead of four 3-partition ones.
"""

import math
import os
import sys
from contextlib import ExitStack

import numpy as np

for _p in ("/opt/trn_rl_repo", "/root/.axon_site/_ro/trn_rl_repo"):
    if os.path.isdir(_p) and _p not in sys.path:
        sys.path.insert(0, _p)

import concourse.bass as bass
import concourse.tile as tile
from concourse import bacc
from concourse import mybir
from concourse.bass_utils import run_bass_kernel_spmd

F32 = mybir.dt.float32
F16 = mybir.dt.float16


def _ensure_axon_hooks():
    """Provide antenv.axon_hooks if the image lacks it, so BASS_TRACE=1
    profiling works (and never crashes) under axon."""
    try:
        import antenv.axon_hooks  # noqa: F401
        return
    except ImportError:
        pass
    try:
        import types

        import antenv

        mod = types.ModuleType("antenv.axon_hooks")
        mod._hook = None
        mod.set_axon_ntff_profile_hook = lambda h: setattr(mod, "_hook", h)
        mod.get_axon_ntff_profile_hook = lambda: mod._hook
        sys.modules["antenv.axon_hooks"] = mod
        antenv.axon_hooks = mod
        from trn_agent_boot.trn_boot import _ntff_profile_via_ctypes

        hook = _ntff_profile_via_ctypes("/opt/axon/libaxon_pjrt.so")
        if hook is not None:
            mod._hook = hook
    except Exception:
        pass


_ensure_axon_hooks()

N_CORES = 8
CHUNK = 512          # points per matmul (free dim; one PSUM bank)
WAVE = 4 * CHUNK     # points per wave (one PSUM set)

_BUILD_CACHE: dict[int, tuple] = {}

# test.py can read timing info from here after a traced run
LAST_RESULT = None


def _build(cap: int):
    """Build the SPMD Bass program for `cap` points per core (multiple of 512)."""
    assert cap % CHUNK == 0
    nchunk = cap // CHUNK
    nw = (nchunk + 3) // 4          # number of waves (last may be partial)
    cap4 = nw * CHUNK               # columns of the [12, cap4] x/out layout

    nc = bacc.Bacc("TRN2", target_bir_lowering=False, debug=False,
                   num_devices=N_CORES)

    # single-DMA layouts keep semaphore fan-in on the first matmul small
    x_d = nc.dram_tensor("x", [128, cap4], F16, kind="ExternalInput").ap()
    wts_d = nc.dram_tensor("wts", [128, 1286], F16, kind="ExternalInput").ap()
    bias_d = nc.dram_tensor("bias", [128, 7], F32, kind="ExternalInput").ap()
    out_d = nc.dram_tensor("out", [12, cap4], F32, kind="ExternalOutput").ap()

    with tile.TileContext(nc) as tc, ExitStack() as ctx:
        consts = ctx.enter_context(tc.tile_pool(name="consts", bufs=1))
        hpools = [ctx.enter_context(tc.tile_pool(name=f"h{l}", bufs=2))
                  for l in range(3)]
        opool = ctx.enter_context(tc.tile_pool(name="osb", bufs=2))
        ppool = ctx.enter_context(tc.tile_pool(name="psum", bufs=2, space="PSUM"))

        # x in the 4-row-group layout: partitions 32j..32j+2 hold chunk 4B+j,
        # one DMA per wave-pair so each adds at most one semaphore to a matmul
        # DMA issue order matters: the Sync queue is serial, so load exactly
        # what the opening phases need first (w1 slice, wave-0 x, biases),
        # then the fat weights, then the remaining x waves.
        x_sb = consts.tile([128, cap4], F16, tag="x_sb")
        wts_sb = consts.tile([128, 1286], F16, tag="wts_sb")
        bias_sb_t = consts.tile([128, 7], F32, tag="bias_sb_t")
        # dummy tanh on a zero scratch: walrus emits the ~2.7us ACT
        # table load before the first Tanh ACTIVATE, so this pulls it into
        # the boot window instead of the first wave's critical path
        warm_a = consts.tile([1, 1], F32, tag="warm_a")
        warm_b = consts.tile([1, 1], F32, tag="warm_b")
        nc.vector.memset(warm_a[:], 0.0)
        nc.scalar.activation(warm_b[:], warm_a[:],
                             mybir.ActivationFunctionType.Tanh)
        nc.sync.dma_start(out=wts_sb[:, 0:256], in_=wts_d[:, 0:256])
        nc.sync.dma_start(out=x_sb[:, 0:CHUNK], in_=x_d[:, 0:CHUNK])
        nc.sync.dma_start(out=bias_sb_t[:], in_=bias_d[:])
        nc.sync.dma_start(out=wts_sb[:, 256:1286], in_=wts_d[:, 256:1286])
        for p0 in range(1, nw):
            sl = slice(p0 * CHUNK, (p0 + 1) * CHUNK)
            nc.sync.dma_start(out=x_sb[:, sl], in_=x_d[:, sl])

        w1_sb = wts_sb[:, 0:256]
        w2_sb = wts_sb[:, 256:768].rearrange("p (k m) -> p k m", k=2)
        w3_sb = wts_sb[:, 768:1280].rearrange("p (k m) -> p k m", k=2)
        w4_sb = wts_sb[:, 1280:1286].rearrange("p (k m) -> p k m", k=2)
        b4_sb = bias_sb_t[:, 6:7]

        w_sb = [w1_sb, w2_sb, w3_sb]
        bias_sb = [bias_sb_t[:, 0:2], bias_sb_t[:, 2:4], bias_sb_t[:, 4:6]]

        def mm(out, lhsT, rhs, **kw):
            nc.tensor.matmul(out, lhsT, rhs, **kw)

        htiles: dict[int, list] = {}

        def hidden_phase(wv, tcnt, lyr, m):
            """Layer lyr (0..2), M-half m of wave wv with tcnt chunks."""
            P = ppool.tile([128, 4, CHUNK], F32, tag="pset")
            for j in range(tcnt):
                if lyr == 0:
                    mm(P[:, j, :],
                       w1_sb[32 * j:32 * j + 3, m * 128:(m + 1) * 128],
                       x_sb[32 * j:32 * j + 3, wv * CHUNK:(wv + 1) * CHUNK],
                       tile_position=(32 * j, 0), start=True, stop=True)
                else:
                    hin = htiles[wv][lyr - 1]
                    for k in range(2):
                        mm(P[:, j, :],
                           w_sb[lyr][:, k, m * 128:(m + 1) * 128],
                           hin[:, k, j, :],
                           start=(k == 0), stop=(k == 1))
            if m == 0:
                htiles[wv][lyr] = hpools[lyr].tile(
                    [128, 2, tcnt, CHUNK], F16,
                    name=f"h{lyr}_{wv}", tag=f"h{lyr}")
            hout = htiles[wv][lyr]
            nc.scalar.activation(hout[:, m, :, :], P[:, 0:tcnt, :],
                                 mybir.ActivationFunctionType.Tanh,
                                 bias=bias_sb[lyr][:, m:m + 1])

        def out_phase(wv, tcnt):
            P = ppool.tile([128, 4, CHUNK], F32, tag="pset")
            h3 = htiles[wv][2]
            # The L4 matmuls only write 12 of the bank's 128 partitions; the
            # rest was fully written by earlier hidden phases (finite data,
            # discarded by the strided out-DMA), so hardware doesn't need
            # the memset -- only CoreSim's cross-tile read check does.
            if os.environ.get("KERNEL_SIM_SAFE"):
                nc.vector.memset(P[:, 0, :], 0.0)
            for j in range(tcnt):
                for k in range(2):
                    mm(P[32 * j:32 * j + 3, 0, :],
                       w4_sb[:, k, :],
                       h3[:, k, j, :],
                       tile_position=(0, 32 * j),
                       start=(k == 0), stop=(k == 1))
            osb = opool.tile([128, CHUNK], F32, tag="osb")
            nc.scalar.activation(osb[:], P[:, 0, :],
                                 mybir.ActivationFunctionType.Tanh,
                                 bias=b4_sb)
            for j in range(tcnt):
                nc.sync.dma_start(
                    out=out_d[3 * j:3 * j + 3, wv * CHUNK:(wv + 1) * CHUNK],
                    in_=osb[32 * j:32 * j + 3, :])

        waves = []
        rem = nchunk
        for wv in range(nw):
            waves.append((wv, min(4, rem)))
            rem -= 4

        # Software-pipeline waves in pairs so ACT never stalls on its own
        # wave, and defer each pair's (PE-heavy, ACT-light) L4 phases into
        # the next pair's opening so ACT keeps draining full hidden sets
        # across pair boundaries.  (Deeper stagger variants and offset wave
        # streams were measured slower: the 2-slot PSUM rotation makes the
        # lockstep pair schedule near-optimal.)
        pending_l4 = []
        i = 0
        while i < len(waves):
            pair = waves[i:i + 2]
            for ph in range(6):
                for wv, tcnt in pair:
                    if ph == 0:
                        htiles[wv] = [None, None, None]
                    hidden_phase(wv, tcnt, ph // 2, ph % 2)
                    if ph <= 1 and pending_l4:
                        lwv, ltcnt = pending_l4.pop(0)
                        out_phase(lwv, ltcnt)
                        del htiles[lwv]
            pending_l4.extend(pair)
            i += 2
        for lwv, ltcnt in pending_l4:
            out_phase(lwv, ltcnt)
            del htiles[lwv]

    nc.compile()
    return nc, nw, cap4


def _get_program(cap: int):
    if cap not in _BUILD_CACHE:
        _BUILD_CACHE[cap] = _build(cap)
    return _BUILD_CACHE[cap]


def _pack_points(pts: np.ndarray, cap: int, nw: int) -> np.ndarray:
    """[cap,3] row-major points -> [128, nw*512] four-row-group layout."""
    nchunk = cap // CHUNK
    a = pts.reshape(nchunk, CHUNK, 3)
    if nchunk < nw * 4:
        pad = np.zeros((nw * 4 - nchunk, CHUNK, 3), np.float32)
        a = np.concatenate([a, pad], axis=0)
    # a[B*4+j, r, i] -> out[32j+i, B*512+r]
    x12 = a.reshape(nw, 4, CHUNK, 3).transpose(1, 3, 0, 2).reshape(4, 3, nw * CHUNK)
    full = np.zeros((128, nw * CHUNK), np.float16)
    for g in range(4):
        full[32 * g:32 * g + 3] = x12[g]
    return full


def _unpack_points(o: np.ndarray, nw: int) -> np.ndarray:
    """[12, nw*512] -> [nw*2048, 3] row-major points."""
    return o.reshape(4, 3, nw, CHUNK).transpose(2, 0, 3, 1).reshape(-1, 3)




def _pack_weights(W1, W2, W3, W4) -> np.ndarray:
    """-> [128, 1286]: w1(row-group replicated) | w2 | w3 | w4, lhsT layouts."""
    wts = np.zeros((128, 1286), np.float16)
    for g in range(4):
        wts[32 * g:32 * g + 3, 0:256] = W1
    wts[:, 256:768] = W2.reshape(2, 128, 256).transpose(1, 0, 2).reshape(128, 512)
    wts[:, 768:1280] = W3.reshape(2, 128, 256).transpose(1, 0, 2).reshape(128, 512)
    wts[:, 1280:1286] = W4.reshape(2, 128, 3).transpose(1, 0, 2).reshape(128, 6)
    return wts


def _pack_biases(b1, b2, b3, b4) -> np.ndarray:
    """-> [128, 7]: b1 (m0,m1) | b2 | b3 | b4 (col-group replicated)."""
    bias = np.zeros((128, 7), np.float32)
    bias[:, 0:2] = b1.reshape(2, 128).T
    bias[:, 2:4] = b2.reshape(2, 128).T
    bias[:, 4:6] = b3.reshape(2, 128).T
    for g in range(4):
        bias[32 * g:32 * g + 3, 6] = b4
    return bias




def kernel(positions, times, W1, b1, W2, b2, W3, b3, W4, b4):
    global LAST_RESULT
    positions = np.ascontiguousarray(np.asarray(positions, dtype=np.float32))
    times_i = np.asarray(times).astype(np.int64)
    W1 = np.asarray(W1, dtype=np.float32)
    W2 = np.asarray(W2, dtype=np.float32)
    W3 = np.asarray(W3, dtype=np.float32)
    W4 = np.asarray(W4, dtype=np.float32)
    b1 = np.asarray(b1, dtype=np.float32)
    b2 = np.asarray(b2, dtype=np.float32)
    b3 = np.asarray(b3, dtype=np.float32)
    b4 = np.asarray(b4, dtype=np.float32)

    n = positions.shape[0]
    order = np.argsort(times_i, kind="stable")
    counts = np.bincount(times_i, minlength=N_CORES)
    offs = np.concatenate([[0], np.cumsum(counts)])
    cap = max(CHUNK, int(math.ceil(counts.max() / CHUNK)) * CHUNK)

    nc, nw, cap4 = _get_program(cap)

    xs = positions[order]
    in_maps = []
    for c in range(N_CORES):
        xc = np.zeros((cap, 3), np.float32)
        xc[:counts[c]] = xs[offs[c]:offs[c + 1]]

        in_maps.append({
            "x": _pack_points(xc, cap, nw),
            "wts": _pack_weights(W1[c], W2[c], W3[c], W4[c]),
            "bias": _pack_biases(b1[c], b2[c], b3[c], b4[c]),
        })

    res = run_bass_kernel_spmd(nc, in_maps, list(range(N_CORES)))
    LAST_RESULT = res

    full = np.zeros((n, 3), np.float32)
    for c in range(N_CORES):
        dec = _unpack_points(res.results[c]["out"], nw)
        full[order[offs[c]:offs[c + 1]]] = dec[:counts[c]]
    return full



# revision 2
# speedup vs baseline: 1.0317x; 1.0317x over previous
"""DNeRF distortion MoE-routing kernel for 8 Trainium2 NeuronCores.

Strategy
--------
`times` partitions the N=131072 points into 8 classes; route on the host:
stable-sort points by class, give class c to NeuronCore c (~16384 points),
each core runs the 4-layer MLP (3->256->256->256->3, tanh) once per point.
Host scatters per-core results back to original order.

Device kernel (identical SPMD program on all 8 cores)
-----------------------------------------------------
The workload is tanh-bound: 771 tanh/point through the scalar (ACT) engine
at 1 elem/lane/cycle would cost ~105us/core.  This kernel splits the tanh
work between ACT (exact table tanh) and the vector engine DVE (a degree-13
odd-polynomial tanh approximation fused into 3 custom DVE ops), overlapping
both with the PE matmul stream:

- Points are processed in waves of 2048 (4 chunks of 512).  Each (layer,
  m-half) phase is split into two chunk-pair "lanes": lane-1 (chunks 2,3)
  drains via ACT, lane-0 (chunks 0,1) partially via DVE.
- DVE pieces per wave: L1 m0/m1 (deg-13) + L2 m0 (deg-9) = 3 of 12, chosen
  to balance ACT ~9.8us/wave vs DVE ~9.8us/wave (PE ~8.3us under both).
- DVE tanh: weights of hidden layers are prescaled by s=c^(1/9 or 1/13) so
  the custom op chain is NTC (bias+clamp+scale, PSUM->fp16), NTQ (square +
  two quadratic factors, 8 ALU stages), NTR (third quadratic * g * A) or a
  stock tensor_mul for deg-9.  ACT pieces undo the prescale for free via
  ACTIVATE's scale immediate.
- Lane-0 layers are software-pipelined one wave late per layer (DVE chain
  ~3.3us/piece), so PE/ACT never wait on DVE within a wave.
- PSUM: 4 half-sets of 2 banks: 2 rotate for ACT drains, 2 for DVE.
"""

import math
import os
import sys
from contextlib import ExitStack

import numpy as np

for _p in ("/opt/trn_rl_repo", "/root/.axon_site/_ro/trn_rl_repo"):
    if os.path.isdir(_p) and _p not in sys.path:
        sys.path.insert(0, _p)

import concourse.bass as bass
import concourse.tile as tile
from concourse import bacc
from concourse import mybir
from concourse.bass_utils import run_bass_kernel_spmd

F32 = mybir.dt.float32
F16 = mybir.dt.float16


def _ensure_axon_hooks():
    """Provide antenv.axon_hooks if the image lacks it, so BASS_TRACE=1
    profiling works (and never crashes) under axon."""
    try:
        import antenv.axon_hooks  # noqa: F401
        return
    except ImportError:
        pass
    try:
        import types

        import antenv

        mod = types.ModuleType("antenv.axon_hooks")
        mod._hook = None
        mod.set_axon_ntff_profile_hook = lambda h: setattr(mod, "_hook", h)
        mod.get_axon_ntff_profile_hook = lambda: mod._hook
        sys.modules["antenv.axon_hooks"] = mod
        antenv.axon_hooks = mod
        from trn_agent_boot.trn_boot import _ntff_profile_via_ctypes

        hook = _ntff_profile_via_ctypes("/opt/axon/libaxon_pjrt.so")
        if hook is not None:
            mod._hook = hook
    except Exception:
        pass


_ensure_axon_hooks()

N_CORES = 8
CHUNK = 512          # points per matmul (free dim; one PSUM bank)
WAVE = 4 * CHUNK     # points per wave

# --- degree-13 tanh approx constants (minimax on [-3.4, 3.4], clamped) ---
# tanh(x) ~ A*(t^2+P1t+Q1)(t^2+P2t+Q2)(t^2+P3t+Q3), A = S13*clip(x,+-X0), t=A^2
S13 = 0.36357593
D13 = dict(P1=-3.699536, Q1=3.595345, P2=-1.994294, Q2=1.800969,
           P3=-0.050413, Q3=0.421285, X0=3.4)
# --- degree-9 constants (minimax on [-3.0, 3.0]) ---
S9 = 0.3866949049092339
D9 = dict(P1=-3.2772642351676513, Q1=3.0534540805012704,
          P2=-0.5486696776661311, Q2=0.8264655087576832, X0=3.0)

# Hidden-layer weights are prescaled by S13 on the host; ACT undoes via
# scale=1/S13.  The deg-9 NTC rescales by S9/S13 via its C3 slot.
W_SCALE = S13
INV_W_SCALE = 1.0 / S13

_BUILD_CACHE: dict[int, tuple] = {}
LAST_RESULT = None

# ---------------------------------------------------------------------------
# Custom DVE ops (registered once per process into concourse.dve_ops)
# ---------------------------------------------------------------------------
_OPS: dict = {}


def _register_dve_ops():
    from concourse import dve_ops
    from concourse.dve_spec import (Spec, Src0, Src1, C0, C1, C2, C3,
                                    minn, maxx, sq, _spill_c3_to_src1,
                                    _has_src1, lower)
    from concourse.dve_uop import DveOpSpec
    if "NTK_CLAMP" in dve_ops._SUB_OPCODE_FOR_NAME:
        for n in ("NTK_CLAMP", "NTK_QUART", "NTK_FIN"):
            _OPS[n] = next(o for o in dve_ops.OPS if o.name == n)
        return

    def ref_ntc(in0, in1, c0, c1, c2):
        s = np.asarray(in1, np.float32).reshape(-1, 1)
        return np.minimum(np.maximum(np.asarray(in0, np.float32) + c0, c1), c2) * s

    def ref_ntq(in0, in1, c0, c1, c2):
        a = np.asarray(in0, np.float32)
        t = a * a
        q2 = np.asarray(in1, np.float32).reshape(-1, 1)
        return ((t + c0) * t + c1) * ((t + c2) * t + q2)

    def ref_ntr(in0, in1, c0, c1, c2):
        a = np.asarray(in0, np.float32)
        t = a * a
        return ((t + c0) * t + c1) * np.asarray(in1, np.float32) * a

    ntc_body = _spill_c3_to_src1(minn(maxx(Src0 + C0, C1), C2) * C3)
    _t1 = sq(Src0)
    ntq_body = _spill_c3_to_src1(((_t1 + C0) * _t1 + C1) * ((_t1 + C2) * _t1 + C3))
    _t2 = sq(Src0)
    ntr_body = (((_t2 + C0) * _t2 + C1) * Src1) * Src0
    specs = [
        ("NTK_CLAMP", Spec(body=ntc_body, reference=ref_ntc)),
        ("NTK_QUART", Spec(body=ntq_body, reference=ref_ntq)),
        ("NTK_FIN", Spec(body=ntr_body, reference=ref_ntr)),
    ]
    for name, spec in specs:
        opcode = dve_ops._CUSTOM_DVE_ROW_BASE + len(dve_ops.OPS)
        shas = {}
        for ver in ("v3", "v4"):
            try:
                uops = lower(spec, ver=ver)
                shas[ver] = DveOpSpec(name=name, opcode=opcode, uops=uops,
                                     rd1_en=_has_src1(spec)).sha(ver)
            except Exception:
                pass
        assert shas, f"custom DVE op {name} failed to lower"
        op = dve_ops.DveOp(name=name, spec=spec, subdim=False, uops_sha=shas)
        dve_ops.OPS.append(op)
        dve_ops._SUB_OPCODE_FOR_NAME[name] = opcode
        dve_ops.CUSTOM_DVE_SPECS[name] = spec
        _OPS[name] = op


_register_dve_ops()

# bias_sb column map
BC_ACT = {0: (0, 1), 1: (2, 3), 2: (4, 5)}   # layer -> (m0 col, m1 col)
BC_B4 = 6
BC_DVE = {0: (7, 8), 1: (9, 10), 2: (11, 12)}  # scaled S13*b
BC_Q2_13 = 13     # Q2 of deg-13
BC_S_13 = 14      # NTC scale for deg-13 pieces: 1.0
BC_S_9 = 15       # NTC scale for deg-9 pieces: S9/S13
BC_Q2_9 = 16      # Q2 of deg-9
BIAS_COLS = 17


def _build(cap: int):
    """Build the SPMD Bass program for `cap` points per core (multiple of 512)."""
    assert cap % CHUNK == 0
    nchunk = cap // CHUNK
    nw = (nchunk + 3) // 4
    cap4 = nw * CHUNK

    nc = bacc.Bacc("TRN2", target_bir_lowering=False, debug=False,
                   num_devices=N_CORES)

    x_d = nc.dram_tensor("x", [128, cap4], F16, kind="ExternalInput").ap()
    wts_d = nc.dram_tensor("wts", [128, 1286], F16, kind="ExternalInput").ap()
    bias_d = nc.dram_tensor("bias", [128, BIAS_COLS], F32, kind="ExternalInput").ap()
    out_d = nc.dram_tensor("out", [12, cap4], F32, kind="ExternalOutput").ap()

    sim_safe = bool(os.environ.get("KERNEL_SIM_SAFE"))

    with tile.TileContext(nc) as tc, ExitStack() as ctx:
        consts = ctx.enter_context(tc.tile_pool(name="consts", bufs=1))
        hpools = [ctx.enter_context(tc.tile_pool(name=f"h{l}", bufs=3))
                  for l in range(3)]
        opool = ctx.enter_context(tc.tile_pool(name="osb", bufs=2))
        pact = ctx.enter_context(tc.tile_pool(name="pact", bufs=2, space="PSUM"))
        pdve = ctx.enter_context(tc.tile_pool(name="pdve", bufs=2, space="PSUM"))
        apool = ctx.enter_context(tc.tile_pool(name="adve", bufs=2))
        gpool = ctx.enter_context(tc.tile_pool(name="gdve", bufs=2))

        x_sb = consts.tile([128, cap4], F16, tag="x_sb")
        wts_sb = consts.tile([128, 1286], F16, tag="wts_sb")
        bias_sb = consts.tile([128, BIAS_COLS], F32, tag="bias_sb")
        # dummy tanh pulls the ~2.7us ACT table load into the boot window
        warm_a = consts.tile([1, 1], F32, tag="warm_a")
        warm_b = consts.tile([1, 1], F32, tag="warm_b")
        nc.vector.memset(warm_a[:], 0.0)
        nc.scalar.activation(warm_b[:], warm_a[:],
                             mybir.ActivationFunctionType.Tanh)
        nc.sync.dma_start(out=wts_sb[:, 0:256], in_=wts_d[:, 0:256])
        nc.sync.dma_start(out=x_sb[:, 0:CHUNK], in_=x_d[:, 0:CHUNK])
        nc.sync.dma_start(out=bias_sb[:], in_=bias_d[:])
        nc.sync.dma_start(out=wts_sb[:, 256:1286], in_=wts_d[:, 256:1286])
        for p0 in range(1, nw):
            sl = slice(p0 * CHUNK, (p0 + 1) * CHUNK)
            nc.sync.dma_start(out=x_sb[:, sl], in_=x_d[:, sl])

        w1_sb = wts_sb[:, 0:256]
        w2_sb = wts_sb[:, 256:768].rearrange("p (k m) -> p k m", k=2)
        w3_sb = wts_sb[:, 768:1280].rearrange("p (k m) -> p k m", k=2)
        w4_sb = wts_sb[:, 1280:1286].rearrange("p (k m) -> p k m", k=2)
        w_sb = [w1_sb, w2_sb, w3_sb]

        def mm(out, lhsT, rhs, **kw):
            nc.tensor.matmul(out, lhsT, rhs, **kw)

        htiles: dict[tuple, object] = {}   # (wave, layer) -> h tile

        def get_h(wv, l):
            key = (wv, l)
            if key not in htiles:
                htiles[key] = hpools[l].tile([128, 2, 4, CHUNK], F16,
                                             name=f"h{l}_{wv}", tag=f"h{l}")
            return htiles[key]

        def fill_hidden(P, wv, l, m, js):
            """Fill PSUM piece P[:, 0:len(js), :] for layer l, m-half m,
            chunks js of wave wv."""
            if l == 0:
                for jj, j in enumerate(js):
                    mm(P[:, jj, :],
                       w1_sb[32 * j:32 * j + 3, m * 128:(m + 1) * 128],
                       x_sb[32 * j:32 * j + 3, wv * CHUNK:(wv + 1) * CHUNK],
                       tile_position=(32 * j, 0), start=True, stop=True)
            else:
                hin = get_h(wv, l - 1)
                for k in range(2):
                    for jj, j in enumerate(js):
                        mm(P[:, jj, :],
                           w_sb[l][:, k, m * 128:(m + 1) * 128],
                           hin[:, k, j, :],
                           start=(k == 0), stop=(k == 1))

        def drain_act(P, wv, l, m, js):
            hout = get_h(wv, l)
            nc.scalar.activation(hout[:, m, js[0]:js[0] + len(js), :],
                                 P[:, 0:len(js), :],
                                 mybir.ActivationFunctionType.Tanh,
                                 bias=bias_sb[:, BC_ACT[l][m]:BC_ACT[l][m] + 1],
                                 scale=INV_W_SCALE)

        def drain_dve(P, wv, l, m, js, deg13):
            n = len(js)
            hout = get_h(wv, l)
            A = apool.tile([128, 2, CHUNK], F16, tag="adve")
            g = gpool.tile([128, 2, CHUNK], F16, tag="gdve")
            D = D13 if deg13 else D9
            S = S13 if deg13 else S9
            scol = BC_S_13 if deg13 else BC_S_9
            qcol = BC_Q2_13 if deg13 else BC_Q2_9
            bcol = BC_DVE[l][m]
            sx0 = W_SCALE * D["X0"]
            nc.vector._custom_dve(_OPS["NTK_CLAMP"], out=A[:, 0:n, :],
                                  in0=P[:, 0:n, :],
                                  in1=bias_sb[:, scol:scol + 1],
                                  s0=bias_sb[:, bcol:bcol + 1],
                                  s1=-sx0, imm2=sx0)
            nc.vector._custom_dve(_OPS["NTK_QUART"], out=g[:, 0:n, :],
                                  in0=A[:, 0:n, :],
                                  in1=bias_sb[:, qcol:qcol + 1],
                                  s0=D["P1"], s1=D["Q1"], imm2=D["P2"])
            hsl = hout[:, m, js[0]:js[0] + n, :]
            if deg13:
                nc.vector._custom_dve(_OPS["NTK_FIN"], out=hsl,
                                      in0=A[:, 0:n, :], in1=g[:, 0:n, :],
                                      s0=D["P3"], s1=D["Q3"])
            else:
                nc.vector.tensor_tensor(hsl, g[:, 0:n, :], A[:, 0:n, :],
                                        mybir.AluOpType.mult)

        def job_hidden(wv, l, m, js, sink, deg13=True):
            pool = pdve if sink == "dve" else pact
            P = pool.tile([128, 2, CHUNK], F32, tag=pool.name)
            fill_hidden(P, wv, l, m, js)
            if sink == "dve":
                drain_dve(P, wv, l, m, js, deg13)
            else:
                drain_act(P, wv, l, m, js)

        def job_out(wv, js):
            P4 = pact.tile([128, 2, CHUNK], F32, tag="pact")
            h3 = get_h(wv, 2)
            if sim_safe:
                nc.vector.memset(P4[:, 0, :], 0.0)
            for k in range(2):
                for j in js:
                    mm(P4[32 * j:32 * j + 3, 0, :],
                       w4_sb[:, k, :],
                       h3[:, k, j, :],
                       tile_position=(0, 32 * j),
                       start=(k == 0), stop=(k == 1))
            osb = opool.tile([128, CHUNK], F32, tag="osb")
            nc.scalar.activation(osb[:], P4[:, 0, :],
                                 mybir.ActivationFunctionType.Tanh,
                                 bias=bias_sb[:, BC_B4:BC_B4 + 1])
            for j in js:
                nc.sync.dma_start(
                    out=out_d[3 * j:3 * j + 3, wv * CHUNK:(wv + 1) * CHUNK],
                    in_=osb[32 * j:32 * j + 3, :])

        # wave list: full waves have 4 chunks; the last may be partial
        waves = []
        rem = nchunk
        for wv in range(nw):
            waves.append((wv, min(4, rem)))
            rem -= 4
        nfull = sum(1 for _, t in waves if t == 4)

        def full(w):
            return 0 <= w < nfull

        # Skewed schedule: lane-1 (chunks 2,3) in-wave via ACT; lane-0
        # (chunks 0,1) one layer per tick via DVE (L1, L2m0) + ACT (rest).
        for t in range(nw + 2):
            # lane1 L1 + lane0 L1 of wave t
            if full(t):
                job_hidden(t, 0, 0, [2, 3], "act")
                job_hidden(t, 0, 1, [2, 3], "act")
                job_hidden(t, 0, 0, [0, 1], "dve", deg13=True)
                job_hidden(t, 0, 1, [0, 1], "dve", deg13=True)
            # partial wave: run its whole (small) chain via ACT at its tick
            if t < nw and waves[t][1] < 4:
                js = list(range(waves[t][1]))
                for l in range(3):
                    for m in range(2):
                        job_hidden(t, l, m, js, "act")
                job_out(t, js)
            # lane1 L2 of wave t
            if full(t):
                job_hidden(t, 1, 0, [2, 3], "act")
                job_hidden(t, 1, 1, [2, 3], "act")
            # lane0 L2 of wave t-1
            if full(t - 1):
                job_hidden(t - 1, 1, 0, [0, 1], "dve", deg13=False)
                job_hidden(t - 1, 1, 1, [0, 1], "act")
            # lane1 L3 of wave t
            if full(t):
                job_hidden(t, 2, 0, [2, 3], "act")
                job_hidden(t, 2, 1, [2, 3], "act")
            # lane0 L3 of wave t-2
            if full(t - 2):
                job_hidden(t - 2, 2, 0, [0, 1], "act")
                job_hidden(t - 2, 2, 1, [0, 1], "act")
                job_out(t - 2, [0, 1, 2, 3])
                del htiles[(t - 2, 2)]

    nc.compile()
    return nc, nw, cap4


def _get_program(cap: int):
    if cap not in _BUILD_CACHE:
        _BUILD_CACHE[cap] = _build(cap)
    return _BUILD_CACHE[cap]


def _pack_points(pts: np.ndarray, cap: int, nw: int) -> np.ndarray:
    """[cap,3] row-major points -> [128, nw*512] four-row-group layout."""
    nchunk = cap // CHUNK
    a = pts.reshape(nchunk, CHUNK, 3)
    if nchunk < nw * 4:
        pad = np.zeros((nw * 4 - nchunk, CHUNK, 3), np.float32)
        a = np.concatenate([a, pad], axis=0)
    x12 = a.reshape(nw, 4, CHUNK, 3).transpose(1, 3, 0, 2).reshape(4, 3, nw * CHUNK)
    full = np.zeros((128, nw * CHUNK), np.float16)
    for g in range(4):
        full[32 * g:32 * g + 3] = x12[g]
    return full


def _unpack_points(o: np.ndarray, nw: int) -> np.ndarray:
    """[12, nw*512] -> [nw*2048, 3] row-major points."""
    return o.reshape(4, 3, nw, CHUNK).transpose(2, 0, 3, 1).reshape(-1, 3)


def _pack_weights(W1, W2, W3, W4) -> np.ndarray:
    """-> [128, 1286] fp16 lhsT layouts; hidden weights prescaled by W_SCALE."""
    wts = np.zeros((128, 1286), np.float16)
    for g in range(4):
        wts[32 * g:32 * g + 3, 0:256] = W1 * W_SCALE
    wts[:, 256:768] = (W2 * W_SCALE).reshape(2, 128, 256).transpose(1, 0, 2).reshape(128, 512)
    wts[:, 768:1280] = (W3 * W_SCALE).reshape(2, 128, 256).transpose(1, 0, 2).reshape(128, 512)
    wts[:, 1280:1286] = W4.reshape(2, 128, 3).transpose(1, 0, 2).reshape(128, 6)
    return wts


def _pack_biases(b1, b2, b3, b4) -> np.ndarray:
    bias = np.zeros((128, BIAS_COLS), np.float32)
    bias[:, 0:2] = b1.reshape(2, 128).T
    bias[:, 2:4] = b2.reshape(2, 128).T
    bias[:, 4:6] = b3.reshape(2, 128).T
    for g in range(4):
        bias[32 * g:32 * g + 3, BC_B4] = b4
    bias[:, 7:9] = b1.reshape(2, 128).T * W_SCALE
    bias[:, 9:11] = b2.reshape(2, 128).T * W_SCALE
    bias[:, 11:13] = b3.reshape(2, 128).T * W_SCALE
    bias[:, BC_Q2_13] = D13["Q2"]
    bias[:, BC_S_13] = 1.0
    bias[:, BC_S_9] = S9 / S13
    bias[:, BC_Q2_9] = D9["Q2"]
    return bias


def kernel(positions, times, W1, b1, W2, b2, W3, b3, W4, b4):
    global LAST_RESULT
    positions = np.ascontiguousarray(np.asarray(positions, dtype=np.float32))
    times_i = np.asarray(times).astype(np.int64)
    W1 = np.asarray(W1, dtype=np.float32)
    W2 = np.asarray(W2, dtype=np.float32)
    W3 = np.asarray(W3, dtype=np.float32)
    W4 = np.asarray(W4, dtype=np.float32)
    b1 = np.asarray(b1, dtype=np.float32)
    b2 = np.asarray(b2, dtype=np.float32)
    b3 = np.asarray(b3, dtype=np.float32)
    b4 = np.asarray(b4, dtype=np.float32)

    n = positions.shape[0]
    order = np.argsort(times_i, kind="stable")
    counts = np.bincount(times_i, minlength=N_CORES)
    offs = np.concatenate([[0], np.cumsum(counts)])
    cap = max(CHUNK, int(math.ceil(counts.max() / CHUNK)) * CHUNK)

    nc, nw, cap4 = _get_program(cap)

    xs = positions[order]
    in_maps = []
    for c in range(N_CORES):
        xc = np.zeros((cap, 3), np.float32)
        xc[:counts[c]] = xs[offs[c]:offs[c + 1]]
        in_maps.append({
            "x": _pack_points(xc, cap, nw),
            "wts": _pack_weights(W1[c], W2[c], W3[c], W4[c]),
            "bias": _pack_biases(b1[c], b2[c], b3[c], b4[c]),
        })

    res = run_bass_kernel_spmd(nc, in_maps, list(range(N_CORES)))
    LAST_RESULT = res

    full = np.zeros((n, 3), np.float32)
    for c in range(N_CORES):
        dec = _unpack_points(res.results[c]["out"], nw)
        full[order[offs[c]:offs[c + 1]]] = dec[:counts[c]]
    return full


# revision 20
# speedup vs baseline: 1.1698x; 1.1339x over previous
"""DNeRF distortion MoE-routing kernel for 8 Trainium2 NeuronCores.

Strategy
--------
`times` partitions the N=131072 points into 8 classes; route on the host:
stable-sort points by class, give class c to NeuronCore c (~16384 points),
each core runs the 4-layer MLP (3->256->256->256->3, tanh) once per point.
Host scatters per-core results back to original order.

Device kernel (identical SPMD program on all 8 cores)
-----------------------------------------------------
The workload is tanh-bound: 771 tanh/point through the scalar (ACT) engine
at 1 elem/lane/cycle would cost ~105us/core.  This kernel splits the tanh
work between ACT (exact table tanh) and the vector engine DVE (a degree-13
odd-polynomial tanh approximation fused into 3 custom DVE ops), overlapping
both with the PE matmul stream:

- Points are processed in waves of 2048 (4 chunks of 512).  Each (layer,
  m-half) phase is split into two chunk-pair "lanes" of [128,2,512]:
  lane-1 (chunks 2,3) drains via ACT; lane-0 (chunks 0,1) of L1 (deg-13)
  and L2-m0 (deg-9) drains via DVE - 3 of 12 pieces per wave, balancing
  ACT ~10.7us/wave vs DVE ~10.6us/wave (PE ~8.5us under both).
- DVE tanh: hidden-layer weights are prescaled by s=c6^(1/13) on the host
  so the custom-op chain needs no extra scale stage: NTK_CLAMP
  (bias+clamp+scale, PSUM fp32 -> SBUF fp16), NTK_QUART (square + two
  quadratic factors, exactly 8 ALU stages), then NTK_FIN (third quadratic
  * g * A, deg-13) or a stock 2x tensor_mul (deg-9).  ACT pieces undo the
  prescale for free via ACTIVATE's scale immediate.
- Wavefront schedule: layer l of wave w runs at tick w+l, so every
  producer-consumer edge crosses a tick boundary and each engine streams
  its per-tick work with no intra-tick fill->drain round trips.  The
  partial last wave runs during the boot ramp (ticks 0-3) where ACT is
  otherwise idle.
- PSUM: one pool of four [128,2,512] half-sets; a half-set is freed by
  its ACT drain or by NTK_CLAMP (the rest of the DVE chain runs from
  SBUF), so rotation never waits on the 3.7us DVE chain.
"""

import math
import os
import sys
from contextlib import ExitStack

import numpy as np

for _p in ("/opt/trn_rl_repo", "/root/.axon_site/_ro/trn_rl_repo"):
    if os.path.isdir(_p) and _p not in sys.path:
        sys.path.insert(0, _p)

import concourse.bass as bass
import concourse.tile as tile
from concourse import bacc
from concourse import mybir
from concourse.bass_utils import run_bass_kernel_spmd

F32 = mybir.dt.float32
F16 = mybir.dt.float16


def _ensure_axon_hooks():
    """Provide antenv.axon_hooks if the image lacks it, so BASS_TRACE=1
    profiling works (and never crashes) under axon."""
    try:
        import antenv.axon_hooks  # noqa: F401
        return
    except ImportError:
        pass
    try:
        import types

        import antenv

        mod = types.ModuleType("antenv.axon_hooks")
        mod._hook = None
        mod.set_axon_ntff_profile_hook = lambda h: setattr(mod, "_hook", h)
        mod.get_axon_ntff_profile_hook = lambda: mod._hook
        sys.modules["antenv.axon_hooks"] = mod
        antenv.axon_hooks = mod
        from trn_agent_boot.trn_boot import _ntff_profile_via_ctypes

        hook = _ntff_profile_via_ctypes("/opt/axon/libaxon_pjrt.so")
        if hook is not None:
            mod._hook = hook
    except Exception:
        pass


_ensure_axon_hooks()

N_CORES = 8
CHUNK = 512          # points per matmul (free dim; one PSUM bank)
WAVE = 4 * CHUNK     # points per wave

# --- degree-13 tanh approx constants (minimax on [-3.4, 3.4], clamped) ---
# tanh(x) ~ A*(t^2+P1t+Q1)(t^2+P2t+Q2)(t^2+P3t+Q3), A = S13*clip(x,+-X0), t=A^2
S13 = 0.36357593
D13 = dict(P1=-3.699536, Q1=3.595345, P2=-1.994294, Q2=1.800969,
           P3=-0.050413, Q3=0.421285, X0=3.4)
# --- degree-9 constants (minimax on [-3.0, 3.0]) ---
S9 = 0.3866949049092339
D9 = dict(P1=-3.2772642351676513, Q1=3.0534540805012704,
          P2=-0.5486696776661311, Q2=0.8264655087576832, X0=3.0)

# Hidden-layer weights are prescaled by S13 on the host; ACT undoes via
# scale=1/S13.  The deg-9 NTC rescales by S9/S13 via its C3 slot.
W_SCALE = S13
INV_W_SCALE = 1.0 / S13

_BUILD_CACHE: dict[int, tuple] = {}
LAST_RESULT = None

# ---------------------------------------------------------------------------
# Custom DVE ops (registered once per process into concourse.dve_ops)
# ---------------------------------------------------------------------------
_OPS: dict = {}


def _register_dve_ops():
    from concourse import dve_ops
    from concourse.dve_spec import (Spec, Src0, Src1, C0, C1, C2, C3,
                                    minn, maxx, sq, _spill_c3_to_src1,
                                    _has_src1, lower)
    from concourse.dve_uop import DveOpSpec
    if "NTK_CLAMP" in dve_ops._SUB_OPCODE_FOR_NAME:
        for n in ("NTK_CLAMP", "NTK_QUART", "NTK_FIN"):
            _OPS[n] = next(o for o in dve_ops.OPS if o.name == n)
        return

    def ref_ntc(in0, in1, c0, c1, c2):
        s = np.asarray(in1, np.float32).reshape(-1, 1)
        return np.minimum(np.maximum(np.asarray(in0, np.float32) + c0, c1), c2) * s

    def ref_ntq(in0, in1, c0, c1, c2):
        a = np.asarray(in0, np.float32)
        t = a * a
        q2 = np.asarray(in1, np.float32).reshape(-1, 1)
        return ((t + c0) * t + c1) * ((t + c2) * t + q2)

    def ref_ntr(in0, in1, c0, c1, c2):
        a = np.asarray(in0, np.float32)
        t = a * a
        return ((t + c0) * t + c1) * np.asarray(in1, np.float32) * a

    ntc_body = _spill_c3_to_src1(minn(maxx(Src0 + C0, C1), C2) * C3)
    _t1 = sq(Src0)
    ntq_body = _spill_c3_to_src1(((_t1 + C0) * _t1 + C1) * ((_t1 + C2) * _t1 + C3))
    _t2 = sq(Src0)
    ntr_body = (((_t2 + C0) * _t2 + C1) * Src1) * Src0
    specs = [
        ("NTK_CLAMP", Spec(body=ntc_body, reference=ref_ntc)),
        ("NTK_QUART", Spec(body=ntq_body, reference=ref_ntq)),
        ("NTK_FIN", Spec(body=ntr_body, reference=ref_ntr)),
    ]
    for name, spec in specs:
        opcode = dve_ops._CUSTOM_DVE_ROW_BASE + len(dve_ops.OPS)
        shas = {}
        for ver in ("v3", "v4"):
            try:
                uops = lower(spec, ver=ver)
                shas[ver] = DveOpSpec(name=name, opcode=opcode, uops=uops,
                                     rd1_en=_has_src1(spec)).sha(ver)
            except Exception:
                pass
        assert shas, f"custom DVE op {name} failed to lower"
        op = dve_ops.DveOp(name=name, spec=spec, subdim=False, uops_sha=shas)
        dve_ops.OPS.append(op)
        dve_ops._SUB_OPCODE_FOR_NAME[name] = opcode
        dve_ops.CUSTOM_DVE_SPECS[name] = spec
        _OPS[name] = op


_register_dve_ops()

# bias_sb column map
BC_ACT = {0: (0, 1), 1: (2, 3), 2: (4, 5)}   # layer -> (m0 col, m1 col)
BC_B4 = 6
BC_DVE = {0: (7, 8), 1: (9, 10), 2: (11, 12)}  # scaled S13*b
BC_Q2_13 = 13     # Q2 of deg-13
BC_S_13 = 14      # NTC scale for deg-13 pieces: 1.0
BC_S_9 = 15       # NTC scale for deg-9 pieces: S9/S13
BC_Q2_9 = 16      # Q2 of deg-9
BIAS_COLS = 17


def _build(cap: int):
    """Build the SPMD Bass program for `cap` points per core (multiple of 512)."""
    assert cap % CHUNK == 0
    nchunk = cap // CHUNK
    nw = (nchunk + 3) // 4
    cap4 = nw * CHUNK

    nc = bacc.Bacc("TRN2", target_bir_lowering=False, debug=False,
                   num_devices=N_CORES)

    x_d = nc.dram_tensor("x", [128, cap4], F16, kind="ExternalInput").ap()
    wts_d = nc.dram_tensor("wts", [128, 1286], F16, kind="ExternalInput").ap()
    bias_d = nc.dram_tensor("bias", [128, BIAS_COLS], F32, kind="ExternalInput").ap()
    out_d = nc.dram_tensor("out", [12, cap4], F32, kind="ExternalOutput").ap()

    sim_safe = bool(os.environ.get("KERNEL_SIM_SAFE"))

    with tile.TileContext(nc) as tc, ExitStack() as ctx:
        consts = ctx.enter_context(tc.tile_pool(name="consts", bufs=1))
        hpools = [ctx.enter_context(tc.tile_pool(name=f"h{l}", bufs=3))
                  for l in range(3)]
        opool = ctx.enter_context(tc.tile_pool(name="osb", bufs=2))
        ppool = ctx.enter_context(tc.tile_pool(name="ppool", bufs=4, space="PSUM"))
        apool = ctx.enter_context(tc.tile_pool(name="adve", bufs=2))
        gpool = ctx.enter_context(tc.tile_pool(name="gdve", bufs=2))

        x_sb = consts.tile([128, cap4], F16, tag="x_sb")
        wts_sb = consts.tile([128, 1286], F16, tag="wts_sb")
        bias_sb = consts.tile([128, BIAS_COLS], F32, tag="bias_sb")
        # dummy tanh pulls the ~2.7us ACT table load into the boot window
        warm_a = consts.tile([1, 1], F32, tag="warm_a")
        warm_b = consts.tile([1, 1], F32, tag="warm_b")
        nc.vector.memset(warm_a[:], 0.0)
        nc.scalar.activation(warm_b[:], warm_a[:],
                             mybir.ActivationFunctionType.Tanh)
        nc.sync.dma_start(out=wts_sb[:, 0:256], in_=wts_d[:, 0:256])
        nc.sync.dma_start(out=x_sb[:, 0:CHUNK], in_=x_d[:, 0:CHUNK])
        nc.sync.dma_start(out=bias_sb[:], in_=bias_d[:])
        # the partial wave (processed during the boot ramp) needs its x early
        xorder = list(range(1, nw))
        if nw * 4 > (cap // CHUNK):  # last wave partial
            xorder.remove(nw - 1)
            xorder.insert(0, nw - 1)
        for p0 in xorder[:2]:
            sl = slice(p0 * CHUNK, (p0 + 1) * CHUNK)
            nc.sync.dma_start(out=x_sb[:, sl], in_=x_d[:, sl])
        nc.sync.dma_start(out=wts_sb[:, 256:768], in_=wts_d[:, 256:768])
        for p0 in xorder[2:]:
            sl = slice(p0 * CHUNK, (p0 + 1) * CHUNK)
            nc.sync.dma_start(out=x_sb[:, sl], in_=x_d[:, sl])
        nc.sync.dma_start(out=wts_sb[:, 768:1286], in_=wts_d[:, 768:1286])

        w1_sb = wts_sb[:, 0:256]
        w2_sb = wts_sb[:, 256:768].rearrange("p (k m) -> p k m", k=2)
        w3_sb = wts_sb[:, 768:1280].rearrange("p (k m) -> p k m", k=2)
        w4_sb = wts_sb[:, 1280:1286].rearrange("p (k m) -> p k m", k=2)
        w_sb = [w1_sb, w2_sb, w3_sb]

        def mm(out, lhsT, rhs, **kw):
            nc.tensor.matmul(out, lhsT, rhs, **kw)

        htiles: dict[tuple, object] = {}   # (wave, layer) -> h tile

        def get_h(wv, l, lane=0):
            key = (wv, l)
            if key not in htiles:
                htiles[key] = hpools[l].tile([128, 2, 4, CHUNK], F16,
                                             name=f"h{l}_{wv}", tag=f"h{l}")
            return htiles[key]

        def fill_hidden(P, wv, l, m, js):
            """Fill PSUM piece P[:, 0:len(js), :] for layer l, m-half m,
            chunks js of wave wv."""
            if l == 0:
                for jj, j in enumerate(js):
                    mm(P[:, jj, :],
                       w1_sb[32 * j:32 * j + 3, m * 128:(m + 1) * 128],
                       x_sb[32 * j:32 * j + 3, wv * CHUNK:(wv + 1) * CHUNK],
                       tile_position=(32 * j, 0), start=True, stop=True)
            else:
                hin = get_h(wv, l - 1)
                for k in range(2):
                    for jj, j in enumerate(js):
                        mm(P[:, jj, :],
                           w_sb[l][:, k, m * 128:(m + 1) * 128],
                           hin[:, k, j, :],
                           start=(k == 0), stop=(k == 1))

        def drain_act(P, wv, l, m, js):
            hout = get_h(wv, l)
            j0 = js[0]
            nc.scalar.activation(hout[:, m, j0:j0 + len(js), :],
                                 P[:, 0:len(js), :],
                                 mybir.ActivationFunctionType.Tanh,
                                 bias=bias_sb[:, BC_ACT[l][m]:BC_ACT[l][m] + 1],
                                 scale=INV_W_SCALE)

        def drain_dve(P, wv, l, m, js, deg13):
            n = len(js)
            hout = get_h(wv, l)
            A = apool.tile([128, 2, CHUNK], F16, tag="adve")
            g = gpool.tile([128, 2, CHUNK], F16, tag="gdve")
            D = D13 if deg13 else D9
            S = S13 if deg13 else S9
            scol = BC_S_13 if deg13 else BC_S_9
            qcol = BC_Q2_13 if deg13 else BC_Q2_9
            bcol = BC_DVE[l][m]
            sx0 = W_SCALE * D["X0"]
            nc.vector._custom_dve(_OPS["NTK_CLAMP"], out=A[:, 0:n, :],
                                  in0=P[:, 0:n, :],
                                  in1=bias_sb[:, scol:scol + 1],
                                  s0=bias_sb[:, bcol:bcol + 1],
                                  s1=-sx0, imm2=sx0)
            nc.vector._custom_dve(_OPS["NTK_QUART"], out=g[:, 0:n, :],
                                  in0=A[:, 0:n, :],
                                  in1=bias_sb[:, qcol:qcol + 1],
                                  s0=D["P1"], s1=D["Q1"], imm2=D["P2"])
            hsl = hout[:, m, js[0]:js[0] + n, :]
            if deg13:
                nc.vector._custom_dve(_OPS["NTK_FIN"], out=hsl,
                                      in0=A[:, 0:n, :], in1=g[:, 0:n, :],
                                      s0=D["P3"], s1=D["Q3"])
            else:
                nc.vector.tensor_tensor(hsl, g[:, 0:n, :], A[:, 0:n, :],
                                        mybir.AluOpType.mult)

        def job_hidden(wv, l, m, js, sink, deg13=True):
            P = ppool.tile([128, 2, CHUNK], F32, tag="ppool")
            fill_hidden(P, wv, l, m, js)
            if sink == "dve":
                drain_dve(P, wv, l, m, js, deg13)
            else:
                drain_act(P, wv, l, m, js)

        def job_out(wv, js):
            P4 = ppool.tile([128, 2, CHUNK], F32, tag="ppool")
            if sim_safe:
                nc.vector.memset(P4[:, 0, :], 0.0)
            h3 = get_h(wv, 2)
            for k in range(2):
                for j in js:
                    mm(P4[32 * j:32 * j + 3, 0, :],
                       w4_sb[:, k, :],
                       h3[:, k, j, :],
                       tile_position=(0, 32 * j),
                       start=(k == 0), stop=(k == 1))
            osb = opool.tile([128, CHUNK], F32, tag="osb")
            nc.scalar.activation(osb[:], P4[:, 0, :],
                                 mybir.ActivationFunctionType.Tanh,
                                 bias=bias_sb[:, BC_B4:BC_B4 + 1])
            for j in js:
                nc.sync.dma_start(
                    out=out_d[3 * j:3 * j + 3, wv * CHUNK:(wv + 1) * CHUNK],
                    in_=osb[32 * j:32 * j + 3, :])

        # wave list: full waves have 4 chunks; the last may be partial
        waves = []
        rem = nchunk
        for wv in range(nw):
            waves.append((wv, min(4, rem)))
            rem -= 4

        def tcnt(w):
            return waves[w][1] if 0 <= w < nw else 0

        # Wavefront schedule: layer l of wave w is processed at tick
        # sched[w][l], normally w+l, so dependencies cross tick boundaries
        # and each engine streams its per-tick work.  Per tick the DVE
        # handles L1 lane-0 (deg-13) + L2 m0 lane-0 (deg-9); ACT the rest.
        # Ramp: the (independent) partial wave and a compressed wave-0
        # chain fill ACT's otherwise-idle boot ticks.
        def emit_hidden(w, l, allow_dve=True):
            tc_ = tcnt(w)
            if tc_ == 0:
                return
            lanes = [[j for j in js if j < tc_]
                     for js in ([0, 1], [2, 3]) if js[0] < tc_]
            for m in range(2):
                for li, js in enumerate(lanes):
                    dve = (allow_dve and tc_ == 4 and li == 0 and
                           (l == 0 or (l == 1 and m == 0)))
                    if dve:
                        job_hidden(w, l, m, js, "dve", deg13=(l == 0))
                    else:
                        job_hidden(w, l, m, js, "act")

        # tick -> list of (wave, stage); stage 0..2 = hidden layer, 3 = out
        sched: dict[int, list] = {}

        def put(t, w, stage):
            sched.setdefault(t, []).append((w, stage))

        partial = [w for w in range(nw) if 0 < tcnt(w) < 4]
        fullw = [w for w in range(nw) if tcnt(w) == 4]
        for i, w in enumerate(fullw):
            for l in range(3):
                put(i + l, w, l)
            put(i + 3, w, 3)
        for w in partial:           # boot-ramp filler: one layer per tick
            for l in range(3):
                put(l, w, l)
            put(3, w, 3)

        no_dve = {fullw[0]} if fullw else set()
        for t in sorted(sched):
            for w, stage in sched[t]:
                if stage < 3:
                    emit_hidden(w, stage, allow_dve=(w not in no_dve))
                else:
                    job_out(w, list(range(tcnt(w))))
                    htiles.pop((w, 2), None)

    nc.compile()
    return nc, nw, cap4


def _get_program(cap: int):
    if cap not in _BUILD_CACHE:
        _BUILD_CACHE[cap] = _build(cap)
    return _BUILD_CACHE[cap]


def _pack_points(pts: np.ndarray, cap: int, nw: int) -> np.ndarray:
    """[cap,3] row-major points -> [128, nw*512] four-row-group layout."""
    nchunk = cap // CHUNK
    a = pts.reshape(nchunk, CHUNK, 3)
    if nchunk < nw * 4:
        pad = np.zeros((nw * 4 - nchunk, CHUNK, 3), np.float32)
        a = np.concatenate([a, pad], axis=0)
    x12 = a.reshape(nw, 4, CHUNK, 3).transpose(1, 3, 0, 2).reshape(4, 3, nw * CHUNK)
    full = np.zeros((128, nw * CHUNK), np.float16)
    for g in range(4):
        full[32 * g:32 * g + 3] = x12[g]
    return full


def _unpack_points(o: np.ndarray, nw: int) -> np.ndarray:
    """[12, nw*512] -> [nw*2048, 3] row-major points."""
    return o.reshape(4, 3, nw, CHUNK).transpose(2, 0, 3, 1).reshape(-1, 3)


def _pack_weights(W1, W2, W3, W4) -> np.ndarray:
    """-> [128, 1286] fp16 lhsT layouts; hidden weights prescaled by W_SCALE."""
    wts = np.zeros((128, 1286), np.float16)
    for g in range(4):
        wts[32 * g:32 * g + 3, 0:256] = W1 * W_SCALE
    wts[:, 256:768] = (W2 * W_SCALE).reshape(2, 128, 256).transpose(1, 0, 2).reshape(128, 512)
    wts[:, 768:1280] = (W3 * W_SCALE).reshape(2, 128, 256).transpose(1, 0, 2).reshape(128, 512)
    wts[:, 1280:1286] = W4.reshape(2, 128, 3).transpose(1, 0, 2).reshape(128, 6)
    return wts


def _pack_biases(b1, b2, b3, b4) -> np.ndarray:
    bias = np.zeros((128, BIAS_COLS), np.float32)
    bias[:, 0:2] = b1.reshape(2, 128).T
    bias[:, 2:4] = b2.reshape(2, 128).T
    bias[:, 4:6] = b3.reshape(2, 128).T
    for g in range(4):
        bias[32 * g:32 * g + 3, BC_B4] = b4
    bias[:, 7:9] = b1.reshape(2, 128).T * W_SCALE
    bias[:, 9:11] = b2.reshape(2, 128).T * W_SCALE
    bias[:, 11:13] = b3.reshape(2, 128).T * W_SCALE
    bias[:, BC_Q2_13] = D13["Q2"]
    bias[:, BC_S_13] = 1.0
    bias[:, BC_S_9] = S9 / S13
    bias[:, BC_Q2_9] = D9["Q2"]
    return bias


def kernel(positions, times, W1, b1, W2, b2, W3, b3, W4, b4):
    global LAST_RESULT
    positions = np.ascontiguousarray(np.asarray(positions, dtype=np.float32))
    times_i = np.asarray(times).astype(np.int64)
    W1 = np.asarray(W1, dtype=np.float32)
    W2 = np.asarray(W2, dtype=np.float32)
    W3 = np.asarray(W3, dtype=np.float32)
    W4 = np.asarray(W4, dtype=np.float32)
    b1 = np.asarray(b1, dtype=np.float32)
    b2 = np.asarray(b2, dtype=np.float32)
    b3 = np.asarray(b3, dtype=np.float32)
    b4 = np.asarray(b4, dtype=np.float32)

    n = positions.shape[0]
    order = np.argsort(times_i, kind="stable")
    counts = np.bincount(times_i, minlength=N_CORES)
    offs = np.concatenate([[0], np.cumsum(counts)])
    cap = max(CHUNK, int(math.ceil(counts.max() / CHUNK)) * CHUNK)

    nc, nw, cap4 = _get_program(cap)

    xs = positions[order]
    in_maps = []
    for c in range(N_CORES):
        xc = np.zeros((cap, 3), np.float32)
        xc[:counts[c]] = xs[offs[c]:offs[c + 1]]
        in_maps.append({
            "x": _pack_points(xc, cap, nw),
            "wts": _pack_weights(W1[c], W2[c], W3[c], W4[c]),
            "bias": _pack_biases(b1[c], b2[c], b3[c], b4[c]),
        })

    res = run_bass_kernel_spmd(nc, in_maps, list(range(N_CORES)))
    LAST_RESULT = res

    full = np.zeros((n, 3), np.float32)
    for c in range(N_CORES):
        dec = _unpack_points(res.results[c]["out"], nw)
        full[order[offs[c]:offs[c + 1]]] = dec[:counts[c]]
    return full


# revision 21
# speedup vs baseline: 1.1883x; 1.0159x over previous
"""DNeRF distortion MoE-routing kernel for 8 Trainium2 NeuronCores.

Strategy
--------
`times` partitions the N=131072 points into 8 classes; route on the host:
stable-sort points by class, give class c to NeuronCore c (~16384 points),
each core runs the 4-layer MLP (3->256->256->256->3, tanh) once per point.
Host scatters per-core results back to original order.

Device kernel (identical SPMD program on all 8 cores)
-----------------------------------------------------
The workload is tanh-bound: 771 tanh/point through the scalar (ACT) engine
at 1 elem/lane/cycle would cost ~105us/core.  This kernel splits the tanh
work between ACT (exact table tanh) and the vector engine DVE (a degree-13
odd-polynomial tanh approximation fused into 3 custom DVE ops), overlapping
both with the PE matmul stream:

- Points are processed in waves of 2048 (4 chunks of 512).  Each (layer,
  m-half) phase is split into two chunk-pair "lanes" of [128,2,512]:
  lane-1 (chunks 2,3) drains via ACT; lane-0 (chunks 0,1) of L1 (deg-13)
  and L2-m0 (deg-9) drains via DVE - 3 of 12 pieces per wave, balancing
  ACT ~10.7us/wave vs DVE ~10.6us/wave (PE ~8.5us under both).
- DVE tanh: hidden-layer weights are prescaled by s=c6^(1/13) on the host
  so the custom-op chain needs no extra scale stage: NTK_CLAMP
  (bias+clamp+scale, PSUM fp32 -> SBUF fp16), NTK_QUART (square + two
  quadratic factors, exactly 8 ALU stages), then NTK_FIN (third quadratic
  * g * A, deg-13) or a stock 2x tensor_mul (deg-9).  ACT pieces undo the
  prescale for free via ACTIVATE's scale immediate.
- Wavefront schedule: layer l of wave w runs at tick w+l, so every
  producer-consumer edge crosses a tick boundary and each engine streams
  its per-tick work with no intra-tick fill->drain round trips.  The
  partial last wave runs during the boot ramp (ticks 0-3) where ACT is
  otherwise idle.
- PSUM: one pool of four [128,2,512] half-sets; a half-set is freed by
  its ACT drain or by NTK_CLAMP (the rest of the DVE chain runs from
  SBUF), so rotation never waits on the 3.7us DVE chain.
"""

import math
import os
import sys
from contextlib import ExitStack

import numpy as np

for _p in ("/opt/trn_rl_repo", "/root/.axon_site/_ro/trn_rl_repo"):
    if os.path.isdir(_p) and _p not in sys.path:
        sys.path.insert(0, _p)

import concourse.bass as bass
import concourse.tile as tile
from concourse import bacc
from concourse import mybir
from concourse.bass_utils import run_bass_kernel_spmd

F32 = mybir.dt.float32
F16 = mybir.dt.float16


def _ensure_axon_hooks():
    """Provide antenv.axon_hooks if the image lacks it, so BASS_TRACE=1
    profiling works (and never crashes) under axon."""
    try:
        import antenv.axon_hooks  # noqa: F401
        return
    except ImportError:
        pass
    try:
        import types

        import antenv

        mod = types.ModuleType("antenv.axon_hooks")
        mod._hook = None
        mod.set_axon_ntff_profile_hook = lambda h: setattr(mod, "_hook", h)
        mod.get_axon_ntff_profile_hook = lambda: mod._hook
        sys.modules["antenv.axon_hooks"] = mod
        antenv.axon_hooks = mod
        from trn_agent_boot.trn_boot import _ntff_profile_via_ctypes

        hook = _ntff_profile_via_ctypes("/opt/axon/libaxon_pjrt.so")
        if hook is not None:
            mod._hook = hook
    except Exception:
        pass


_ensure_axon_hooks()

N_CORES = 8
CHUNK = 512          # points per matmul (free dim; one PSUM bank)
WAVE = 4 * CHUNK     # points per wave

# --- degree-13 tanh approx constants (minimax on [-3.4, 3.4], clamped) ---
# tanh(x) ~ A*(t^2+P1t+Q1)(t^2+P2t+Q2)(t^2+P3t+Q3), A = S13*clip(x,+-X0), t=A^2
S13 = 0.36357593
D13 = dict(P1=-3.699536, Q1=3.595345, P2=-1.994294, Q2=1.800969,
           P3=-0.050413, Q3=0.421285, X0=3.4)
# --- degree-9 constants (minimax on [-3.0, 3.0]) ---
S9 = 0.3866949049092339
D9 = dict(P1=-3.2772642351676513, Q1=3.0534540805012704,
          P2=-0.5486696776661311, Q2=0.8264655087576832, X0=3.0)

# Hidden-layer weights are prescaled by S13 on the host; ACT undoes via
# scale=1/S13.  The deg-9 NTC rescales by S9/S13 via its C3 slot.
W_SCALE = S13
INV_W_SCALE = 1.0 / S13

_BUILD_CACHE: dict[int, tuple] = {}
LAST_RESULT = None

# ---------------------------------------------------------------------------
# Custom DVE ops (registered once per process into concourse.dve_ops)
# ---------------------------------------------------------------------------
_OPS: dict = {}


def _register_dve_ops():
    from concourse import dve_ops
    from concourse.dve_spec import (Spec, Src0, Src1, C0, C1, C2, C3,
                                    minn, maxx, sq, _spill_c3_to_src1,
                                    _has_src1, lower)
    from concourse.dve_uop import DveOpSpec
    if "NTK_CLAMP" in dve_ops._SUB_OPCODE_FOR_NAME:
        for n in ("NTK_CLAMP", "NTK_QUART", "NTK_FIN"):
            _OPS[n] = next(o for o in dve_ops.OPS if o.name == n)
        return

    def ref_ntc(in0, in1, c0, c1, c2):
        s = np.asarray(in1, np.float32).reshape(-1, 1)
        return np.minimum(np.maximum(np.asarray(in0, np.float32) + c0, c1), c2) * s

    def ref_ntq(in0, in1, c0, c1, c2):
        a = np.asarray(in0, np.float32)
        t = a * a
        q2 = np.asarray(in1, np.float32).reshape(-1, 1)
        return ((t + c0) * t + c1) * ((t + c2) * t + q2)

    def ref_ntr(in0, in1, c0, c1, c2):
        a = np.asarray(in0, np.float32)
        t = a * a
        return ((t + c0) * t + c1) * np.asarray(in1, np.float32) * a

    ntc_body = _spill_c3_to_src1(minn(maxx(Src0 + C0, C1), C2) * C3)
    _t1 = sq(Src0)
    ntq_body = _spill_c3_to_src1(((_t1 + C0) * _t1 + C1) * ((_t1 + C2) * _t1 + C3))
    _t2 = sq(Src0)
    ntr_body = (((_t2 + C0) * _t2 + C1) * Src1) * Src0
    specs = [
        ("NTK_CLAMP", Spec(body=ntc_body, reference=ref_ntc)),
        ("NTK_QUART", Spec(body=ntq_body, reference=ref_ntq)),
        ("NTK_FIN", Spec(body=ntr_body, reference=ref_ntr)),
    ]
    for name, spec in specs:
        opcode = dve_ops._CUSTOM_DVE_ROW_BASE + len(dve_ops.OPS)
        shas = {}
        for ver in ("v3", "v4"):
            try:
                uops = lower(spec, ver=ver)
                shas[ver] = DveOpSpec(name=name, opcode=opcode, uops=uops,
                                     rd1_en=_has_src1(spec)).sha(ver)
            except Exception:
                pass
        assert shas, f"custom DVE op {name} failed to lower"
        op = dve_ops.DveOp(name=name, spec=spec, subdim=False, uops_sha=shas)
        dve_ops.OPS.append(op)
        dve_ops._SUB_OPCODE_FOR_NAME[name] = opcode
        dve_ops.CUSTOM_DVE_SPECS[name] = spec
        _OPS[name] = op


_register_dve_ops()

# bias_sb column map
BC_ACT = {0: (0, 1), 1: (2, 3), 2: (4, 5)}   # layer -> (m0 col, m1 col)
BC_B4 = 6
BC_DVE = {0: (7, 8), 1: (9, 10), 2: (11, 12)}  # scaled S13*b
BC_Q2_13 = 13     # Q2 of deg-13
BC_S_13 = 14      # NTC scale for deg-13 pieces: 1.0
BC_S_9 = 15       # NTC scale for deg-9 pieces: S9/S13
BC_Q2_9 = 16      # Q2 of deg-9
BIAS_COLS = 17


def _build(cap: int):
    """Build the SPMD Bass program for `cap` points per core (multiple of 512)."""
    assert cap % CHUNK == 0
    nchunk = cap // CHUNK
    nw = (nchunk + 3) // 4
    cap4 = nw * CHUNK

    nc = bacc.Bacc("TRN2", target_bir_lowering=False, debug=False,
                   num_devices=N_CORES)

    x_d = nc.dram_tensor("x", [128, cap4], F16, kind="ExternalInput").ap()
    wts_d = nc.dram_tensor("wts", [128, 1286], F16, kind="ExternalInput").ap()
    bias_d = nc.dram_tensor("bias", [128, BIAS_COLS], F32, kind="ExternalInput").ap()
    out_d = nc.dram_tensor("out", [12, cap4], F32, kind="ExternalOutput").ap()

    sim_safe = bool(os.environ.get("KERNEL_SIM_SAFE"))

    with tile.TileContext(nc) as tc, ExitStack() as ctx:
        consts = ctx.enter_context(tc.tile_pool(name="consts", bufs=1))
        hpools = [ctx.enter_context(tc.tile_pool(name=f"h{l}", bufs=3))
                  for l in range(3)]
        opool = ctx.enter_context(tc.tile_pool(name="osb", bufs=2))
        ppool = ctx.enter_context(tc.tile_pool(name="ppool", bufs=4, space="PSUM"))
        apool = ctx.enter_context(tc.tile_pool(name="adve", bufs=2))
        gpool = ctx.enter_context(tc.tile_pool(name="gdve", bufs=2))

        x_sb = consts.tile([128, cap4], F16, tag="x_sb")
        wts_sb = consts.tile([128, 1286], F16, tag="wts_sb")
        bias_sb = consts.tile([128, BIAS_COLS], F32, tag="bias_sb")
        # dummy tanh pulls the ~2.7us ACT table load into the boot window
        warm_a = consts.tile([1, 1], F32, tag="warm_a")
        warm_b = consts.tile([1, 1], F32, tag="warm_b")
        nc.vector.memset(warm_a[:], 0.0)
        nc.scalar.activation(warm_b[:], warm_a[:],
                             mybir.ActivationFunctionType.Tanh)
        nc.sync.dma_start(out=wts_sb[:, 0:256], in_=wts_d[:, 0:256])
        nc.sync.dma_start(out=x_sb[:, 0:CHUNK], in_=x_d[:, 0:CHUNK])
        nc.sync.dma_start(out=bias_sb[:], in_=bias_d[:])
        # the partial wave (processed during the boot ramp) needs its x early
        xorder = list(range(1, nw))
        if nw * 4 > (cap // CHUNK):  # last wave partial
            xorder.remove(nw - 1)
            xorder.insert(0, nw - 1)
        for p0 in xorder[:2]:
            sl = slice(p0 * CHUNK, (p0 + 1) * CHUNK)
            nc.sync.dma_start(out=x_sb[:, sl], in_=x_d[:, sl])
        nc.sync.dma_start(out=wts_sb[:, 256:768], in_=wts_d[:, 256:768])
        for p0 in xorder[2:]:
            sl = slice(p0 * CHUNK, (p0 + 1) * CHUNK)
            nc.sync.dma_start(out=x_sb[:, sl], in_=x_d[:, sl])
        nc.sync.dma_start(out=wts_sb[:, 768:1286], in_=wts_d[:, 768:1286])

        w1_sb = wts_sb[:, 0:256]
        w2_sb = wts_sb[:, 256:768].rearrange("p (k m) -> p k m", k=2)
        w3_sb = wts_sb[:, 768:1280].rearrange("p (k m) -> p k m", k=2)
        w4_sb = wts_sb[:, 1280:1286].rearrange("p (k m) -> p k m", k=2)
        w_sb = [w1_sb, w2_sb, w3_sb]

        def mm(out, lhsT, rhs, **kw):
            nc.tensor.matmul(out, lhsT, rhs, **kw)

        htiles: dict[tuple, object] = {}   # (wave, layer) -> h tile

        def get_h(wv, l, lane=0):
            key = (wv, l)
            if key not in htiles:
                htiles[key] = hpools[l].tile([128, 2, 4, CHUNK], F16,
                                             name=f"h{l}_{wv}", tag=f"h{l}")
            return htiles[key]

        def fill_hidden(P, wv, l, m, js):
            """Fill PSUM piece P[:, 0:len(js), :] for layer l, m-half m,
            chunks js of wave wv."""
            if l == 0:
                for jj, j in enumerate(js):
                    mm(P[:, jj, :],
                       w1_sb[32 * j:32 * j + 3, m * 128:(m + 1) * 128],
                       x_sb[32 * j:32 * j + 3, wv * CHUNK:(wv + 1) * CHUNK],
                       tile_position=(32 * j, 0), start=True, stop=True)
            else:
                hin = get_h(wv, l - 1)
                for k in range(2):
                    for jj, j in enumerate(js):
                        mm(P[:, jj, :],
                           w_sb[l][:, k, m * 128:(m + 1) * 128],
                           hin[:, k, j, :],
                           start=(k == 0), stop=(k == 1))

        def drain_act(P, wv, l, m, js):
            hout = get_h(wv, l)
            j0 = js[0]
            nc.scalar.activation(hout[:, m, j0:j0 + len(js), :],
                                 P[:, 0:len(js), :],
                                 mybir.ActivationFunctionType.Tanh,
                                 bias=bias_sb[:, BC_ACT[l][m]:BC_ACT[l][m] + 1],
                                 scale=INV_W_SCALE)

        def drain_dve(P, wv, l, m, js, deg13):
            n = len(js)
            hout = get_h(wv, l)
            A = apool.tile([128, 2, CHUNK], F16, tag="adve")
            g = gpool.tile([128, 2, CHUNK], F16, tag="gdve")
            D = D13 if deg13 else D9
            S = S13 if deg13 else S9
            scol = BC_S_13 if deg13 else BC_S_9
            qcol = BC_Q2_13 if deg13 else BC_Q2_9
            bcol = BC_DVE[l][m]
            sx0 = W_SCALE * D["X0"]
            nc.vector._custom_dve(_OPS["NTK_CLAMP"], out=A[:, 0:n, :],
                                  in0=P[:, 0:n, :],
                                  in1=bias_sb[:, scol:scol + 1],
                                  s0=bias_sb[:, bcol:bcol + 1],
                                  s1=-sx0, imm2=sx0)
            nc.vector._custom_dve(_OPS["NTK_QUART"], out=g[:, 0:n, :],
                                  in0=A[:, 0:n, :],
                                  in1=bias_sb[:, qcol:qcol + 1],
                                  s0=D["P1"], s1=D["Q1"], imm2=D["P2"])
            hsl = hout[:, m, js[0]:js[0] + n, :]
            if deg13:
                nc.vector._custom_dve(_OPS["NTK_FIN"], out=hsl,
                                      in0=A[:, 0:n, :], in1=g[:, 0:n, :],
                                      s0=D["P3"], s1=D["Q3"])
            else:
                nc.vector.tensor_tensor(hsl, g[:, 0:n, :], A[:, 0:n, :],
                                        mybir.AluOpType.mult)

        def job_hidden(wv, l, m, js, sink, deg13=True):
            P = ppool.tile([128, 2, CHUNK], F32, tag="ppool")
            fill_hidden(P, wv, l, m, js)
            if sink == "dve":
                drain_dve(P, wv, l, m, js, deg13)
            else:
                drain_act(P, wv, l, m, js)

        def job_out(wv, js):
            P4 = ppool.tile([128, 2, CHUNK], F32, tag="ppool")
            if sim_safe:
                nc.vector.memset(P4[:, 0, :], 0.0)
            h3 = get_h(wv, 2)
            for k in range(2):
                for j in js:
                    mm(P4[32 * j:32 * j + 3, 0, :],
                       w4_sb[:, k, :],
                       h3[:, k, j, :],
                       tile_position=(0, 32 * j),
                       start=(k == 0), stop=(k == 1))
            osb = opool.tile([128, CHUNK], F32, tag="osb")
            nc.scalar.activation(osb[:], P4[:, 0, :],
                                 mybir.ActivationFunctionType.Tanh,
                                 bias=bias_sb[:, BC_B4:BC_B4 + 1])
            for j in js:
                nc.sync.dma_start(
                    out=out_d[3 * j:3 * j + 3, wv * CHUNK:(wv + 1) * CHUNK],
                    in_=osb[32 * j:32 * j + 3, :])

        # wave list: full waves have 4 chunks; the last may be partial
        waves = []
        rem = nchunk
        for wv in range(nw):
            waves.append((wv, min(4, rem)))
            rem -= 4

        def tcnt(w):
            return waves[w][1] if 0 <= w < nw else 0

        # Wavefront schedule: layer l of wave w is processed at tick
        # sched[w][l], normally w+l, so dependencies cross tick boundaries
        # and each engine streams its per-tick work.  Per tick the DVE
        # handles L1 lane-0 (deg-13) + L2 m0 lane-0 (deg-9); ACT the rest.
        # Ramp: the (independent) partial wave and a compressed wave-0
        # chain fill ACT's otherwise-idle boot ticks.
        def emit_hidden(w, l, allow_dve=True):
            tc_ = tcnt(w)
            if tc_ == 0:
                return
            lanes = [[j for j in js if j < tc_]
                     for js in ([0, 1], [2, 3]) if js[0] < tc_]
            for m in range(2):
                # ACT lane first: any conservative same-tile WAW edge then
                # points DVE-after-ACT (cheap) instead of ACT-after-NTR
                for li in reversed(range(len(lanes))):
                    js = lanes[li]
                    dve = (allow_dve and tc_ == 4 and li == 0 and
                           (l == 0 or (l == 1 and m == 0)))
                    if dve:
                        job_hidden(w, l, m, js, "dve", deg13=(l == 0))
                    else:
                        job_hidden(w, l, m, js, "act")

        # tick -> list of (wave, stage); stage 0..2 = hidden layer, 3 = out
        sched: dict[int, list] = {}

        def put(t, w, stage):
            sched.setdefault(t, []).append((w, stage))

        partial = [w for w in range(nw) if 0 < tcnt(w) < 4]
        fullw = [w for w in range(nw) if tcnt(w) == 4]
        for i, w in enumerate(fullw):
            for l in range(3):
                put(i + l, w, l)
            put(i + 3, w, 3)
        for w in partial:           # boot-ramp filler: one layer per tick
            for l in range(3):
                put(l, w, l)
            put(3, w, 3)

        no_dve = {fullw[0]} if fullw else set()
        for t in sorted(sched):
            for w, stage in sched[t]:
                if stage < 3:
                    emit_hidden(w, stage, allow_dve=(w not in no_dve))
                else:
                    job_out(w, list(range(tcnt(w))))
                    htiles.pop((w, 2), None)

    nc.compile()
    return nc, nw, cap4


def _get_program(cap: int):
    if cap not in _BUILD_CACHE:
        _BUILD_CACHE[cap] = _build(cap)
    return _BUILD_CACHE[cap]


def _pack_points(pts: np.ndarray, cap: int, nw: int) -> np.ndarray:
    """[cap,3] row-major points -> [128, nw*512] four-row-group layout."""
    nchunk = cap // CHUNK
    a = pts.reshape(nchunk, CHUNK, 3)
    if nchunk < nw * 4:
        pad = np.zeros((nw * 4 - nchunk, CHUNK, 3), np.float32)
        a = np.concatenate([a, pad], axis=0)
    x12 = a.reshape(nw, 4, CHUNK, 3).transpose(1, 3, 0, 2).reshape(4, 3, nw * CHUNK)
    full = np.zeros((128, nw * CHUNK), np.float16)
    for g in range(4):
        full[32 * g:32 * g + 3] = x12[g]
    return full


def _unpack_points(o: np.ndarray, nw: int) -> np.ndarray:
    """[12, nw*512] -> [nw*2048, 3] row-major points."""
    return o.reshape(4, 3, nw, CHUNK).transpose(2, 0, 3, 1).reshape(-1, 3)


def _pack_weights(W1, W2, W3, W4) -> np.ndarray:
    """-> [128, 1286] fp16 lhsT layouts; hidden weights prescaled by W_SCALE."""
    wts = np.zeros((128, 1286), np.float16)
    for g in range(4):
        wts[32 * g:32 * g + 3, 0:256] = W1 * W_SCALE
    wts[:, 256:768] = (W2 * W_SCALE).reshape(2, 128, 256).transpose(1, 0, 2).reshape(128, 512)
    wts[:, 768:1280] = (W3 * W_SCALE).reshape(2, 128, 256).transpose(1, 0, 2).reshape(128, 512)
    wts[:, 1280:1286] = W4.reshape(2, 128, 3).transpose(1, 0, 2).reshape(128, 6)
    return wts


def _pack_biases(b1, b2, b3, b4) -> np.ndarray:
    bias = np.zeros((128, BIAS_COLS), np.float32)
    bias[:, 0:2] = b1.reshape(2, 128).T
    bias[:, 2:4] = b2.reshape(2, 128).T
    bias[:, 4:6] = b3.reshape(2, 128).T
    for g in range(4):
        bias[32 * g:32 * g + 3, BC_B4] = b4
    bias[:, 7:9] = b1.reshape(2, 128).T * W_SCALE
    bias[:, 9:11] = b2.reshape(2, 128).T * W_SCALE
    bias[:, 11:13] = b3.reshape(2, 128).T * W_SCALE
    bias[:, BC_Q2_13] = D13["Q2"]
    bias[:, BC_S_13] = 1.0
    bias[:, BC_S_9] = S9 / S13
    bias[:, BC_Q2_9] = D9["Q2"]
    return bias


def kernel(positions, times, W1, b1, W2, b2, W3, b3, W4, b4):
    global LAST_RESULT
    positions = np.ascontiguousarray(np.asarray(positions, dtype=np.float32))
    times_i = np.asarray(times).astype(np.int64)
    W1 = np.asarray(W1, dtype=np.float32)
    W2 = np.asarray(W2, dtype=np.float32)
    W3 = np.asarray(W3, dtype=np.float32)
    W4 = np.asarray(W4, dtype=np.float32)
    b1 = np.asarray(b1, dtype=np.float32)
    b2 = np.asarray(b2, dtype=np.float32)
    b3 = np.asarray(b3, dtype=np.float32)
    b4 = np.asarray(b4, dtype=np.float32)

    n = positions.shape[0]
    order = np.argsort(times_i, kind="stable")
    counts = np.bincount(times_i, minlength=N_CORES)
    offs = np.concatenate([[0], np.cumsum(counts)])
    cap = max(CHUNK, int(math.ceil(counts.max() / CHUNK)) * CHUNK)

    nc, nw, cap4 = _get_program(cap)

    xs = positions[order]
    in_maps = []
    for c in range(N_CORES):
        xc = np.zeros((cap, 3), np.float32)
        xc[:counts[c]] = xs[offs[c]:offs[c + 1]]
        in_maps.append({
            "x": _pack_points(xc, cap, nw),
            "wts": _pack_weights(W1[c], W2[c], W3[c], W4[c]),
            "bias": _pack_biases(b1[c], b2[c], b3[c], b4[c]),
        })

    res = run_bass_kernel_spmd(nc, in_maps, list(range(N_CORES)))
    LAST_RESULT = res

    full = np.zeros((n, 3), np.float32)
    for c in range(N_CORES):
        dec = _unpack_points(res.results[c]["out"], nw)
        full[order[offs[c]:offs[c + 1]]] = dec[:counts[c]]
    return full
